# revision 25
# baseline (speedup 1.0000x reference)
"""Trainium2 Bass kernel for nn_DSVDD (retrieval_knn) - fp8 DoubleRow version.

Math (per batch b):
  phi = W @ p_b + bias            [DIM, HW]    (1x1 conv)
  sqdist[i,j] = ||phi_i||^2 + ||C_j||^2 - 2 phi_i . C_j
  top-3 smallest distances d0<=d1<=d2  ->  w0 = 1/(1+exp(d0-d1)+exp(d0-d2))
  score[i] = w0 * d0

Device strategy (8 cores, data-parallel over (batch, HW-half)):
  All matmuls run as fp8e4 DoubleRow (K=256 per instruction) with 448-wide
  moving streams so the ~320-cycle weight loads hide completely.  Host
  pre-scales into fp8 range: p*4, W*64 so the conv PSUM holds 256*phi; ACT
  re-quantizes phi to fp8(8*phi) and squares for f (one 448-wide ACT pair
  per conv block).  Prototype bank cb = fp8(64*C) so the G-phase PSUM
  holds 256*(2 phi.C) over 7 exact 448-wide j-slices; DVE adds the
  replicated -256*||c_j||^2 row in place on PSUM, then max8 reads PSUM
  directly into a candidate buffer (8 per slice); one final max8 per
  i-tile ranks the 56 candidates.  Tail avoids Exp entirely (2nd-order
  softmin expansion, gaps ~1e-2) and uses a single batched Sqrt.
  Startup DMAs ride three independent queues (sync/gpsimd/scalar).
"""
import sys

sys.path.insert(0, "/opt/trn_rl_repo")

import numpy as np
import ml_dtypes

B, DIM, H, W_ = 4, 1792, 56, 56
HW = H * W_            # 3136
P = 3136               # prototypes
NCORES = 8
HALF = HW // 2         # 1568 positions per core
KC = DIM // 128        # 14 contraction chunks
KCP = KC // 2          # 7 DoubleRow pairs
BLKS = [(0, 448), (448, 448), (896, 336), (1232, 336)]   # conv i-blocks
# (all blocks >= ~320-cycle fp8 DoubleRow weight load, so WL stays hidden)
NIT = 13               # i-tiles: 12 full + 1 ragged(32)
LAST_W = HALF - 12 * 128   # 32
SY = 256.0             # PSUM scale of (2 phi.C)
JW = 448               # G j-slice width (7 * 448 = 3136 exactly)
NJS = 7
NCAND = NJS * 8        # 56 candidates per row
N_WARM = 30

_cache = {}


def _build_program():
    import concourse.tile as tile
    from concourse import bacc, mybir

    F32 = mybir.dt.float32
    F32R = mybir.dt.float32r
    F8 = mybir.dt.float8e4
    BF16 = mybir.dt.bfloat16
    AF = mybir.ActivationFunctionType
    ALU = mybir.AluOpType
    DR = mybir.MatmulPerfMode.DoubleRow

    nc = bacc.Bacc("TRN2", target_bir_lowering=False, debug=False)

    pT8a_d = nc.dram_tensor("pT8a", [128, KC * 448], F8, kind="ExternalInput")
    pT8b_d = nc.dram_tensor("pT8b", [128, KC * 448], F8, kind="ExternalInput")
    pT8c_d = nc.dram_tensor("pT8c", [128, KC * 336], F8, kind="ExternalInput")
    pT8d_d = nc.dram_tensor("pT8d", [128, KC * 336], F8, kind="ExternalInput")
    wt8a_d = nc.dram_tensor("wt8a", [128, 2 * KC * 128], F8,
                            kind="ExternalInput")
    wt8b_d = nc.dram_tensor("wt8b", [128, 12 * KC * 128], F8,
                            kind="ExternalInput")
    cb8a_d = nc.dram_tensor("cb8a", [128, KC * 1568], F8, kind="ExternalInput")
    cb8b_d = nc.dram_tensor("cb8b", [128, KC * 1568], F8, kind="ExternalInput")
    cbcr_d = nc.dram_tensor("cbcr", [128, P], F32, kind="ExternalInput")
    onecb_d = nc.dram_tensor("onecb", [128, 1], BF16, kind="ExternalInput")
    oner_d = nc.dram_tensor("oner", [1, 128], F32R, kind="ExternalInput")
    bias_sq_d = nc.dram_tensor("bias_sq", [128, KC], F32, kind="ExternalInput")
    bias_ph_d = nc.dram_tensor("bias_ph", [128, KC], F32, kind="ExternalInput")
    score_d = nc.dram_tensor("score", [128, NIT], F32, kind="ExternalOutput")

    with tile.TileContext(nc) as tc:
        with (
            tc.tile_pool(name="persist", bufs=1) as persist,
        ):
            phi = persist.tile([128, KC, HALF], F8)
            cb = persist.tile([128, KC, P], F8)
            cbcr = persist.tile([128, P], F32)
            onecb = persist.tile([128, 1], BF16)
            oner = persist.tile([1, 128], F32R)
            bias_sq = persist.tile([128, KC], F32)
            bias_ph = persist.tile([128, KC], F32)
            f_row = persist.tile([1, HALF], F32)
            f_col = persist.tile([128, NIT], F32)
            cand = persist.tile([128, NIT, NCAND], F32)
            top8s = persist.tile([128, NIT, 8], F32)
            aa = persist.tile([128, NIT], F32)
            bb = persist.tile([128, NIT], F32)
            s0t = persist.tile([128, NIT], F32)
            d0t = persist.tile([128, NIT], F32)
            rrt = persist.tile([128, NIT], F32)
            u1t = persist.tile([128, NIT], F32)
            u2t = persist.tile([128, NIT], F32)
            q1t = persist.tile([128, NIT], F32)
            q2t = persist.tile([128, NIT], F32)
            sut = persist.tile([128, NIT], F32)
            wrt = persist.tile([128, NIT], F32)
            score_col = persist.tile([128, NIT], F32)
            scr = persist.tile([128, 2], F32)

            # ------------- conv phase: phi = W @ p + b, f = ||phi||^2 -------
            with (
                tc.tile_pool(name="convp", bufs=1) as convp,
                tc.tile_pool(name="sqp", bufs=12) as sqp,
                tc.tile_pool(name="cps", bufs=4, space="PSUM") as cps,
                tc.tile_pool(name="fps", bufs=1, space="PSUM") as fps,
            ):
                pq = convp.tile([128, KC, HALF], F8)
                wt = convp.tile([128, KC * KC, 128], F8)
                warm = convp.tile([128, 512], F32R)

                # one f accumulator bank per conv i-block
                f_banks = [fps.tile([1, bw], F32, name=f"fp{k}", tag=f"f{k}")
                           for k, (_, bw) in enumerate(BLKS)]

                # startup-critical loads, assigned by measured queue
                # dispatch latency (sync ~8us, scalar ~9.3us, gpsimd ~12.3us)
                nc.sync.dma_start(
                    pq[:, :, 0:448],
                    pT8a_d.rearrange("q (cc i) -> q cc i", cc=KC))
                nc.scalar.dma_start(
                    wt[:, 0:2 * KC, :],
                    wt8a_d.rearrange("q (g d) -> q g d", d=128))
                nc.scalar.dma_start(
                    pq[:, :, 448:896],
                    pT8b_d.rearrange("q (cc i) -> q cc i", cc=KC))
                nc.scalar.dma_start(
                    pq[:, :, 896:1232],
                    pT8c_d.rearrange("q (cc i) -> q cc i", cc=KC))
                nc.gpsimd.dma_start(bias_ph[:], bias_ph_d[:])
                nc.gpsimd.dma_start(bias_sq[:], bias_sq_d[:])
                nc.gpsimd.dma_start(onecb[:], onecb_d[:])
                nc.gpsimd.dma_start(oner[:], oner_d[:])
                nc.gpsimd.dma_start(
                    pq[:, :, 1232:1568],
                    pT8d_d.rearrange("q (cc i) -> q cc i", cc=KC))
                nc.gpsimd.dma_start(
                    wt[:, 2 * KC:, :],
                    wt8b_d.rearrange("q (g d) -> q g d", d=128))
                nc.gpsimd.dma_start(cbcr[:], cbcr_d[:])
                nc.scalar.dma_start(
                    cb[:, :, 0:1568],
                    cb8a_d.rearrange("q (cc j) -> q cc j", cc=KC))
                nc.scalar.dma_start(
                    cb[:, :, 1568:3136],
                    cb8b_d.rearrange("q (cc j) -> q cc j", cc=KC))

                # PE warmup: dummy matmuls keep HAM's activity monitor hot
                # while the first real DMAs land, so conv starts at 2.4 GHz.
                nc.vector.memset(warm[:].bitcast(F32), 1.0)
                for _ in range(N_WARM):
                    wps = cps.tile([128, 448], F32, tag="acc")
                    nc.tensor.matmul(wps[:], warm[:, 0:128], warm[:, 0:448],
                                     start=True, stop=True)

                pending_f = []
                for dcg in range(KC):
                    for k, (i0, bw) in enumerate(BLKS):
                        isl = slice(i0, i0 + bw)
                        acc = cps.tile([128, 448], F32, tag="acc")
                        for cp in range(KCP):
                            nc.tensor.matmul(
                                acc[0:128, 0:bw],
                                wt[:, dcg * KC + 2 * cp:dcg * KC + 2 * cp + 2, :],
                                pq[:, 2 * cp:2 * cp + 2, isl],
                                start=(cp == 0),
                                stop=(cp == KCP - 1),
                                perf_mode=DR,
                            )
                        # deferred f matmuls, flushed in batches so the
                        # PE only rarely switches fp8-DR <-> bf16 pipelines
                        if len(pending_f) >= 8:
                            for args, kw in pending_f:
                                nc.tensor.matmul(*args, **kw)
                            pending_f = []
                        # phi (fp8, scaled 8x) = (psum/256 + b) * 8
                        nc.scalar.activation(
                            phi[:, dcg, isl], acc[0:128, 0:bw], AF.Identity,
                            bias=bias_ph[:, dcg:dcg + 1], scale=1.0 / 32.0,
                        )
                        # phi2 = (psum/256 + b)^2  (bf16)
                        sq = sqp.tile([128, 448], BF16)
                        nc.scalar.activation(
                            sq[0:128, 0:bw], acc[0:128, 0:bw], AF.Square,
                            bias=bias_sq[:, dcg:dcg + 1], scale=1.0 / 256.0,
                        )
                        pending_f.append((
                            (f_banks[k][:], onecb[:], sq[0:128, 0:bw]),
                            dict(start=(dcg == 0), stop=(dcg == KC - 1)),
                        ))
                for args, kw in pending_f:
                    nc.tensor.matmul(*args, **kw)
                pending_f = []
                for k, (i0, bw) in enumerate(BLKS):
                    nc.vector.tensor_copy(f_row[:, i0:i0 + bw], f_banks[k][:])
                # preload the Sqrt ACT table now; no other ACT function runs
                # until the tail, so it stays resident
                nc.scalar.activation(scr[:, 0:1], bias_sq[:, 0:1], AF.Sqrt)

            # ------------- G phase: Y = 256*2phi.C in PSUM, -cn on DVE ------
            # (f relayout is folded in after the first j-slice so the PE
            # never waits on the DVE f_row copies.)
            with (
                tc.tile_pool(name="yps", bufs=7, space="PSUM") as yps,
                tc.tile_pool(name="ftp", bufs=1, space="PSUM") as ftp,
            ):
                for js in range(NJS):
                    j0 = js * JW
                    for it in range(NIT):
                        w = 128 if it < 12 else LAST_W
                        i0 = it * 128
                        y = yps.tile([128, JW], F32, name="y", tag="y")
                        for cp in range(KCP):
                            nc.tensor.matmul(
                                y[0:w, :],
                                phi[:, 2 * cp:2 * cp + 2, i0:i0 + w],
                                cb[:, 2 * cp:2 * cp + 2, j0:j0 + JW],
                                start=(cp == 0),
                                stop=(cp == KCP - 1),
                                perf_mode=DR,
                            )
                        # fold in -256*||c_j||^2 in place, then rank
                        nc.vector.tensor_tensor(
                            y[0:w, :], y[0:w, :],
                            cbcr[0:w, j0:j0 + JW], ALU.add,
                        )
                        nc.vector.max(cand[0:w, it, js * 8:(js + 1) * 8],
                                      y[0:w, :])
                        if js == NJS - 1:
                            nc.vector.max(top8s[0:w, it, :], cand[0:w, it, :])
                    if js == 0:
                        # f relayout: [1, 1568] -> [128, 13]
                        ft = ftp.tile([128, NIT], F32)
                        for it in range(NIT):
                            w = 128 if it < 12 else LAST_W
                            nc.tensor.transpose(
                                ft[0:w, it:it + 1],
                                f_row[:, it * 128:it * 128 + w],
                                oner[0:1, 0:1].bitcast(F32),
                            )
                        nc.scalar.activation(f_col[:], ft[:], AF.Copy)

                # ------------- tail: exp-free softmin -----------------------
                t0 = top8s[:, :, 0]
                t1 = top8s[:, :, 1]
                t2 = top8s[:, :, 2]
                TT = nc.vector.tensor_tensor
                TS = nc.vector.tensor_scalar
                TT(aa[:], t0, t1, ALU.subtract)          # t0-t1 >= 0 (Y units)
                TT(bb[:], t0, t2, ALU.subtract)
                TS(s0t[:], t0, -1.0 / SY, None, ALU.mult)
                TT(s0t[:], s0t[:], f_col[:], ALU.add)    # s0 = f - t0/SY
                nc.scalar.activation(d0t[:], s0t[:], AF.Sqrt)
                nc.vector.reciprocal(rrt[:], d0t[:])
                TS(aa[:], aa[:], 1.0 / (2.0 * SY), None, ALU.mult)
                TS(bb[:], bb[:], 1.0 / (2.0 * SY), None, ALU.mult)
                TT(u1t[:], aa[:], rrt[:], ALU.mult)      # u1 ~ d1-d0
                TT(u2t[:], bb[:], rrt[:], ALU.mult)      # u2 ~ d2-d0
                TT(q1t[:], u1t[:], u1t[:], ALU.mult)
                TT(q2t[:], u2t[:], u2t[:], ALU.mult)
                TT(q1t[:], q1t[:], q2t[:], ALU.add)
                TS(q1t[:], q1t[:], 0.5, None, ALU.mult)
                TT(sut[:], u1t[:], u2t[:], ALU.add)
                TT(q1t[:], q1t[:], sut[:], ALU.subtract)
                TS(q1t[:], q1t[:], 3.0, None, ALU.add)
                nc.vector.reciprocal(wrt[:], q1t[:])
                TT(score_col[:], d0t[:], wrt[:], ALU.mult)
                nc.sync.dma_start(score_d[:], score_col[:])

    nc.compile()
    return nc


def _get_program():
    if "nc" not in _cache:
        _cache["nc"] = _build_program()
    return _cache["nc"]


def kernel(p, W, b, C):
    from concourse.bass_utils import run_bass_kernel_spmd

    nc = _get_program()

    F8NP = ml_dtypes.float8_e4m3
    BF16NP = ml_dtypes.bfloat16

    p = np.ascontiguousarray(np.asarray(p, dtype=np.float32))
    W = np.asarray(W, dtype=np.float32)
    b = np.ascontiguousarray(np.asarray(b, dtype=np.float32))
    C = np.ascontiguousarray(np.asarray(C, dtype=np.float32))

    # weights: wt8[q, dcg, cc, d] = 64*W[dcg*128+d, cc*128+q]
    A = (64.0 * W).reshape(KC, 128, KC, 128)           # [dcg, d, cc, q]
    wt8 = np.ascontiguousarray(
        A.transpose(3, 0, 2, 1).reshape(128, KC * KC * 128)).astype(F8NP)
    wt8a = np.ascontiguousarray(wt8[:, 0:2 * KC * 128])
    wt8b = np.ascontiguousarray(wt8[:, 2 * KC * 128:])

    # prototype bank: cb8[q, cc, j] = 64*C[cc*128+q, j]
    cb8 = (64.0 * C).reshape(KC, 128, P).transpose(1, 0, 2).astype(F8NP)
    cb8a = np.ascontiguousarray(cb8[:, :, 0:1568]).reshape(128, KC * 1568)
    cb8b = np.ascontiguousarray(cb8[:, :, 1568:]).reshape(128, KC * 1568)

    cn = np.sum(C.astype(np.float64) * C, axis=0).astype(np.float32)
    cbcr = np.ascontiguousarray(np.broadcast_to(
        (-SY * cn).astype(np.float32)[None, :], (128, P)))

    onecb = np.ones((128, 1), dtype=BF16NP)
    oner = np.ones((1, 128), dtype=np.float32)
    bias_sq = np.ascontiguousarray(b.reshape(KC, 128).T)
    bias_ph = np.ascontiguousarray(8.0 * b.reshape(KC, 128).T)

    p_flat = p.reshape(B, DIM, HW)
    in_maps = []
    for core in range(NCORES):
        bidx, half = divmod(core, 2)
        pT = 4.0 * p_flat[bidx, :, half * HALF:(half + 1) * HALF]
        pq = pT.reshape(KC, 128, HALF).transpose(1, 0, 2)  # [q, cc, i]
        pq8 = pq.astype(F8NP)
        pT8a = np.ascontiguousarray(pq8[:, :, 0:448]).reshape(128, KC * 448)
        pT8b = np.ascontiguousarray(pq8[:, :, 448:896]).reshape(128, KC * 448)
        pT8c = np.ascontiguousarray(
            pq8[:, :, 896:1232]).reshape(128, KC * 336)
        pT8d = np.ascontiguousarray(
            pq8[:, :, 1232:]).reshape(128, KC * 336)
        in_maps.append({
            "pT8a": pT8a, "pT8b": pT8b, "pT8c": pT8c, "pT8d": pT8d,
            "wt8a": wt8a, "wt8b": wt8b,
            "cb8a": cb8a, "cb8b": cb8b, "cbcr": cbcr,
            "onecb": onecb, "oner": oner, "bias_sq": bias_sq,
            "bias_ph": bias_ph,
        })

    _cache["last_in_maps"] = in_maps
    res = run_bass_kernel_spmd(nc, in_maps, list(range(NCORES)))
    _cache["last_result"] = res

    return assemble_output(per_core=[res.results[c]["score"] for c in range(NCORES)])


def assemble_output(per_core=None, res_concat=None):
    if per_core is None:
        sc_all = res_concat["score"]                              # [8*128, 13]
        per_core = [sc_all[c * 128:(c + 1) * 128] for c in range(NCORES)]
    out = np.empty((B, 1, H, W_), dtype=np.float32)
    for core in range(NCORES):
        bidx, half = divmod(core, 2)
        sc = per_core[core]                                       # [128, 13]
        flat = np.empty(HALF, dtype=np.float32)
        flat[:12 * 128] = sc[:, :12].T.reshape(-1)
        flat[12 * 128:] = sc[:LAST_W, 12]
        out.reshape(B, 1, HW)[bidx, 0, half * HALF:(half + 1) * HALF] = flat
    return out
